# revision 9
# baseline (speedup 1.0000x reference)
"""BiLSTM-CRF Trainium2 kernel, v2.

Sharding: 8 cores = 2 LSTM directions x 4 batch quarters. Each core runs
ONE direction for 16 sentences (as 2 independent 8-sentence chains so the
two chains' engine work overlaps). The embedding gather happens on host.

All per-core inputs travel in ONE packed f32 tensor (per-array transfer
setup over the PJRT tunnel costs ~60-100ms, so array count matters as
much as bytes): x sits in the first XW columns as bitcast fp16, the LSTM
weights / Wout / h0 / c0 follow in f32, and bias+bout ride as a [128, 9]
block that one DMA unpacks onto a single partition. x is fp16 on the wire
only; on device it is converted once to f32 and the recurrence runs in
fp32r so the recurrent h feedback adds no quantization error beyond the
fp16 input rounding.

Device layout ("gates-transposed"): gates live as [128 j, batch] so
  - matmul lhsT = weight block [128k, 128j] (stationary), rhs = x/h
    column [128k, 8b] (moving, 8 columns -> tiny per-matmul cost)
  - h is produced directly in [h-on-partitions, b] form: no per-step PE
    transpose, and elementwise ACT/DVE ops use all 128 lanes.
Per step per chain: 32 tiny matmuls (bias rank-1 + x + h1 + h2 per j-tile),
sigmoid + tanh ACT (PSUM->SBUF), 4 DVE ops for the c/h update; h written
straight into hAll which both feeds the next step's matmuls and the bulk
emissions matmul at the end (feats = h @ WoutT + bout, f32 out, K padded
to 10: fp32r needs an even moving size). Viterbi runs on host.

Gate rows are host-permuted to [i, f, o, g]; j-tiles 0..7 map to
i0 i1 f0 f1 o0 o1 g0 g1.
"""

import numpy as np
from contextlib import ExitStack

import concourse.bass as bass
import concourse.bacc as bacc
import concourse.tile as tile
from concourse import mybir
from concourse.bass_utils import run_bass_kernel_spmd

B, T, V, E, H, K = 64, 512, 50000, 128, 256, 9
NCORES = 8
SPC = 16                  # sentences per core (one direction)
NCH = 2                   # independent chains per core
CB = SPC // NCH           # 8 sentences per chain
NJT = 8                   # j tiles (4H / 128)
KP = 10                   # K padded to even (fp32r moving size must be even)
F32 = mybir.dt.float32
F32R = mybir.dt.float32r
F16 = mybir.dt.float16


def _aux_layout(nsteps):
    XW = NCH * CB * nsteps // 2          # fp16 x region, in f32 columns
    off = {"x": 0, "w": XW, "wout": XW + 1536}
    off["h0"] = off["wout"] + 2 * KP
    off["c0"] = off["h0"] + NCH * 2 * CB
    off["bs"] = off["c0"] + NCH * 2 * CB
    end = off["bs"] + 9
    return off, end + (end % 2)


def _build_nc(nsteps=T):
    TS = nsteps
    NTOK = CB * TS
    NBLK = NTOK // 128
    off, AUXW = _aux_layout(TS)
    nc = bacc.Bacc()
    aux_d = nc.dram_tensor("aux", [128, AUXW], F32R, kind="ExternalInput")
    feats_d = nc.dram_tensor("feats", [128, NCH, NBLK, K], F32,
                             kind="ExternalOutput")

    with tile.TileContext(nc) as tc, ExitStack() as ctx:
        const = ctx.enter_context(tc.tile_pool(name="const", bufs=1))
        state = ctx.enter_context(tc.tile_pool(name="state", bufs=1))

        xT = state.tile([128, NCH, NTOK], F32R)
        w_sb = const.tile([128, 3, NJT * 128], F32R)
        with tc.tile_pool(name="wcvt", bufs=1) as wcvt:
            wh = wcvt.tile([128, 3, NJT * 128], F16)
            nc.sync.dma_start(
                out=wh,
                in_=aux_d[:, off["w"]:off["w"] + 1536].bitcast(F16))
            nc.vector.tensor_copy(out=w_sb[:], in_=wh[:])
        wout_sb = const.tile([128, 2, KP], F32R)
        nc.sync.dma_start(out=wout_sb,
                          in_=aux_d[:, off["wout"]:off["wout"] + 2 * KP])
        # bias+bout land on one partition: host packs the [128, 9] aux
        # block in DMA iteration order so biasRow[0, jt*128+p] = bias j,
        # biasRow[0, 1024:1024+KP] = bout
        biasRow = const.tile([1, 9 * 128], F32R)
        nc.sync.dma_start(out=biasRow, in_=aux_d[:, off["bs"]:off["bs"] + 9])
        ones_f32 = const.tile([1, 128], F32)
        nc.vector.memset(ones_f32[:], 1.0)
        ones_sb = ones_f32[:].bitcast(F32R)

        with tc.tile_pool(name="cvt", bufs=1) as cvt:
            xh = cvt.tile([128, NCH, NTOK], F16)
            nc.sync.dma_start(
                out=xh, in_=aux_d[:, off["x"]:off["x"] + NTOK].bitcast(F16))
            # convert in chunks so step 0 isn't gated on the whole stream
            NCVT = 8
            for i in range(NCVT):
                cs = NTOK // NCVT
                nc.vector.tensor_copy(
                    out=xT[:, :, i * cs:(i + 1) * cs],
                    in_=xh[:, :, i * cs:(i + 1) * cs])

        hAll = state.tile([128, NCH, 2, (TS + 1) * CB], F32R)
        nc.sync.dma_start(out=hAll[:, :, :, 0:CB],
                          in_=aux_d[:, off["h0"]:off["h0"] + NCH * 2 * CB])
        c_st = state.tile([128, NCH, 2, CB], F32R)
        nc.sync.dma_start(out=c_st,
                          in_=aux_d[:, off["c0"]:off["c0"] + NCH * 2 * CB])
        feats_sb = state.tile([128, NCH, NBLK, KP], F32)

        gp_ctx = ExitStack()
        gp_pool = gp_ctx.enter_context(
            tc.tile_pool(name="gp", bufs=2, space="PSUM"))
        tmp_pool = ctx.enter_context(tc.tile_pool(name="tmp", bufs=4))

        def step(iv):
            g = []
            for ch in range(NCH):
                # i/f/o gates and g gate in separate PSUM regions so the
                # sigmoid can fire as soon as its 24 matmuls close instead
                # of waiting for the whole 32-matmul group
                gi_ps = gp_pool.tile([128, 6, CB], F32, space="PSUM",
                                     tag=f"gi{ch}", padded_shape=[128, 8, 64],
                                     name=f"gi{ch}")
                gg_ps = gp_pool.tile([128, 2, CB], F32, space="PSUM",
                                     tag=f"gg{ch}", padded_shape=[128, 2, 256],
                                     name=f"gg{ch}")
                for jt in range(NJT):
                    o_ps = gi_ps[:, jt, :] if jt < 6 else gg_ps[:, jt - 6, :]
                    nc.tensor.matmul(
                        out=o_ps,
                        lhsT=biasRow[:, jt * 128:(jt + 1) * 128],
                        rhs=ones_sb[:, 0:CB],
                        start=(jt == 0 or jt == 6), stop=False)
                    nc.tensor.matmul(
                        out=o_ps,
                        lhsT=w_sb[:, 0, jt * 128:(jt + 1) * 128],
                        rhs=xT[:, ch, bass.ts(iv, CB)],
                        start=False, stop=False)
                g.append((gi_ps, gg_ps))
            for ch in range(NCH):
                gi_ps, gg_ps = g[ch]
                for jt in range(NJT):
                    o_ps = gi_ps[:, jt, :] if jt < 6 else gg_ps[:, jt - 6, :]
                    nc.tensor.matmul(
                        out=o_ps,
                        lhsT=w_sb[:, 1, jt * 128:(jt + 1) * 128],
                        rhs=hAll[:, ch, 0, bass.ts(iv, CB)],
                        start=False, stop=False)
                    nc.tensor.matmul(
                        out=o_ps,
                        lhsT=w_sb[:, 2, jt * 128:(jt + 1) * 128],
                        rhs=hAll[:, ch, 1, bass.ts(iv, CB)],
                        start=False, stop=(jt == 5 or jt == NJT - 1))
                sg = tmp_pool.tile([128, 6, CB], F32R, tag=f"sg{ch}")
                nc.scalar.activation(
                    out=sg[:], in_=gi_ps[:, :, :],
                    func=mybir.ActivationFunctionType.Sigmoid)
                tg = tmp_pool.tile([128, 2, CB], F32R, tag=f"tg{ch}")
                nc.scalar.activation(
                    out=tg[:], in_=gg_ps[:, :, :],
                    func=mybir.ActivationFunctionType.Tanh)
                t1 = tmp_pool.tile([128, 2, CB], F32R, tag=f"t1{ch}")
                t2 = tmp_pool.tile([128, 2, CB], F32R, tag=f"t2{ch}")
                nc.vector.tensor_mul(t1[:], sg[:, 2:4, :], c_st[:, ch, :, :])
                nc.vector.tensor_mul(t2[:], sg[:, 0:2, :], tg[:])
                nc.vector.tensor_add(c_st[:, ch, :, :], t1[:], t2[:])
                th = tmp_pool.tile([128, 2, CB], F32R, tag=f"th{ch}")
                nc.scalar.activation(
                    out=th[:], in_=c_st[:, ch, :, :],
                    func=mybir.ActivationFunctionType.Tanh)
                nc.vector.tensor_mul(
                    hAll[:, ch, :, bass.ts(iv + 1, CB)],
                    sg[:, 4:6, :], th[:])

        tc.For_i_unrolled(0, TS, 1, step, max_unroll=8)

        gp_ctx.close()
        fp_ctx = ExitStack()
        fp_pool = fp_ctx.enter_context(
            tc.tile_pool(name="fp", bufs=2, space="PSUM"))
        for ch in range(NCH):
            for blk in range(NBLK):
                f_ps = fp_pool.tile([128, KP], F32, space="PSUM", tag="f",
                                    padded_shape=[128, 512])
                nc.tensor.matmul(
                    out=f_ps[:], lhsT=ones_sb[:, 0:128],
                    rhs=biasRow[:, 1024:1024 + KP], start=True, stop=False)
                off2 = CB + blk * 128
                nc.tensor.matmul(
                    out=f_ps[:], lhsT=hAll[:, ch, 0, off2:off2 + 128],
                    rhs=wout_sb[:, 0, :], start=False, stop=False)
                nc.tensor.matmul(
                    out=f_ps[:], lhsT=hAll[:, ch, 1, off2:off2 + 128],
                    rhs=wout_sb[:, 1, :], start=False, stop=True)
                nc.vector.tensor_copy(out=feats_sb[:, ch, blk, :],
                                      in_=f_ps[:])

        fp_ctx.close()
        nc.sync.dma_start(out=feats_d[:, :, :, :],
                          in_=feats_sb[:, :, :, 0:K])
    nc.compile()
    return nc


_NC_CACHE = None


def _get_nc():
    global _NC_CACHE
    if _NC_CACHE is None:
        _NC_CACHE = _build_nc()
    return _NC_CACHE


# gate-row permutation: torch order (i,f,g,o) -> kernel order (i,f,o,g)
_PERM = np.concatenate([np.arange(0, 512), np.arange(768, 1024),
                        np.arange(512, 768)])


def _prep_inputs(sentence, emb, Wih_f, Whh_f, bih_f, bhh_f,
                 Wih_b, Whh_b, bih_b, bhh_b, Wout, bout, h0, c0):
    # fp16(emb)[tok] == fp16(emb[tok]): casting first halves the
    # gather/transpose traffic
    emb16 = np.asarray(emb).astype(np.float16)
    sent = np.asarray(sentence)
    Wout = np.asarray(Wout, np.float32)
    off, AUXW = _aux_layout(T)
    per_dir = []
    for d, (Wih, Whh, bih, bhh) in enumerate(
            [(Wih_f, Whh_f, bih_f, bhh_f), (Wih_b, Whh_b, bih_b, bhh_b)]):
        Wih = np.asarray(Wih, np.float32)[_PERM]      # [1024, 128]
        Whh = np.asarray(Whh, np.float32)[_PERM]      # [1024, 256]
        w = np.empty((128, 3, NJT * 128), np.float16)
        w[:, 0] = Wih.T
        w[:, 1] = Whh.T[0:128]
        w[:, 2] = Whh.T[128:256]
        bias = ((np.asarray(bih, np.float32) + np.asarray(bhh, np.float32))
                [_PERM]).astype(np.float32)
        wout = np.zeros((128, 2, KP), np.float32)
        wout[:, :, :K] = np.ascontiguousarray(
            Wout[:, d * H:(d + 1) * H].T).reshape(2, 128, K).transpose(1, 0, 2)
        flat = np.zeros(128 * 9, np.float32)
        flat[0:1024] = bias
        if d == 0:
            flat[1024:1024 + K] = np.asarray(bout, np.float32)
        per_dir.append((w, wout, flat.reshape(128, 9)))
    h0 = np.asarray(h0, np.float32)
    c0 = np.asarray(c0, np.float32)

    in_maps = []
    for c in range(NCORES):
        d = c // 4
        sl = slice((c % 4) * SPC, (c % 4) * SPC + SPC)
        x = emb16[sent[sl]]                           # [16, T, E] f16
        if d == 1:
            x = x[:, ::-1, :]
        # xT[e, ch, t*8+b] = x[ch*8+b, t, e]
        xT = np.ascontiguousarray(
            x.reshape(NCH, CB, T, E).transpose(3, 0, 2, 1)
        ).reshape(128, NCH * T * CB)
        h0T = np.ascontiguousarray(
            h0[d, sl].reshape(NCH, CB, 2, 128).transpose(3, 0, 2, 1)
        ).reshape(128, NCH * 2 * CB)
        c0T = np.ascontiguousarray(
            c0[d, sl].reshape(NCH, CB, 2, 128).transpose(3, 0, 2, 1)
        ).reshape(128, NCH * 2 * CB)
        w, wout, bsm = per_dir[d]
        aux = np.empty((128, AUXW), np.float32)
        aux[:, off["bs"] + 9:] = 0.0
        aux[:, off["x"]:off["x"] + NCH * T * CB // 2] = xT.view(np.float32)
        aux[:, off["w"]:off["w"] + 1536] = \
            w.reshape(128, 3072).view(np.float32)
        aux[:, off["wout"]:off["wout"] + 2 * KP] = wout.reshape(128, 2 * KP)
        aux[:, off["h0"]:off["h0"] + NCH * 2 * CB] = h0T
        aux[:, off["c0"]:off["c0"] + NCH * 2 * CB] = c0T
        aux[:, off["bs"]:off["bs"] + 9] = bsm
        in_maps.append({"aux": aux})
    return in_maps


def _feats_from_results(results, nsteps=T):
    """results[c]["feats"] [128, NCH, NBLK, KP] -> feats [B, T, K] f32."""
    NBLK = CB * nsteps // 128
    feats = np.zeros((B, nsteps, K), np.float32)
    for c in range(NCORES):
        d = c // 4
        sl = slice((c % 4) * SPC, (c % 4) * SPC + SPC)
        f = np.asarray(results[c]["feats"]).astype(np.float32)
        # token blk*128+p = t*CB+b  ->  [ch, t, b, K] -> [ch*CB+b, t, K]
        f = f.transpose(1, 2, 0, 3).reshape(NCH, nsteps, CB, K) \
            .transpose(0, 2, 1, 3).reshape(SPC, nsteps, K)
        if d == 1:
            f = f[:, ::-1, :]
        feats[sl] += f
    return feats


def _viterbi_host(feats, mask, start, end, trans):
    """Exact port of the reference viterbi. feats [B,T,K] f32."""
    Bn, Tn, Kn = feats.shape
    score = start[None] + feats[:, 0]
    hist = np.zeros((Tn - 1, Bn, Kn), np.int64)
    for t in range(1, Tn):
        br = score[:, :, None] + trans[None]
        idx = br.argmax(1)
        nxt = np.take_along_axis(br, idx[:, None, :], 1)[:, 0] + feats[:, t]
        score = np.where(mask[:, t][:, None], nxt, score)
        hist[t - 1] = idx
    score = score + end[None]
    tag = score.argmax(-1)
    tags = np.zeros((Bn, Tn), np.int64)
    tags[:, Tn - 1] = tag
    for t in range(Tn - 2, -1, -1):
        tag = np.take_along_axis(hist[t], tag[:, None], 1)[:, 0]
        tags[:, t] = tag
    return tags.astype(np.int32)


def kernel_run(trace=False, **inputs):
    nc = _get_nc()
    in_maps = _prep_inputs(
        inputs["sentence"], inputs["emb"],
        inputs["Wih_f"], inputs["Whh_f"], inputs["bih_f"], inputs["bhh_f"],
        inputs["Wih_b"], inputs["Whh_b"], inputs["bih_b"], inputs["bhh_b"],
        inputs["Wout"], inputs["bout"], inputs["h0"], inputs["c0"])
    res = run_bass_kernel_spmd(nc, in_maps, list(range(NCORES)), trace=trace)
    feats = _feats_from_results(res.results)
    tags = _viterbi_host(feats, np.asarray(inputs["mask"]),
                         np.asarray(inputs["start"], np.float32),
                         np.asarray(inputs["end"], np.float32),
                         np.asarray(inputs["trans"], np.float32))
    return tags, res


def kernel(**inputs):
    tags, _ = kernel_run(trace=False, **inputs)
    return tags
